# revision 1
# baseline (speedup 1.0000x reference)
"""Trainium2 Bass kernel for CrossModalAttention.

Reference computation (per sample n, data-parallel over 8 cores):
  img_mean[c,t]  = mean_v image[c,t,v]
  Q[r,t]         = w_iq @ img_mean + b_iq
  K[r,t]         = w_mq @ (block-mean of motion) + b_mq
  Vv[c,t,v]      = w_iv @ image + b_iv
  s[r,i,j]       = tanh(K[r,i] - Q[r,j])
  att[c,i,j]     = (w_att @ s + b_att)/T1 + I
  y[c,i,v]       = sum_j att[c,i,j] * Vv[c,j,v]

Kernel strategy per core (one sample):
 - stream image per (t, half): PE matmul Vv (fp32r full-rate), gpsimd reduce
   for img_sum, ACT copy PSUM->SBUF with fused b_iv, then SBUF->SBUF "fold"
   DMA into V_fold layout [p=16*cl+j, g*784+v] with channel c = 32*cl+g.
 - tiny attention chain -> att matrix per channel, scattered via DMA into a
   block-diagonal lhsT buffer (32 groups of 8 channels).
 - 32 block-diagonal matmuls y_g = att_bd_g.T @ V_fold_g (full 128-K PE use),
   DVE copy PSUM->SBUF, DMA out with scatter to the natural y layout.
"""

import numpy as np

N, C, T1, H, W = 8, 256, 16, 28, 28
HW = H * W
T2, V = 64, 25
REL = 32
CH = 128  # partition half of C
NCORES = 8
NG = 32  # channel groups (c = 32*cl + g)

_CACHE = {}


def _build():
    from contextlib import ExitStack
    from concourse import bass, mybir, tile, bacc, masks

    dt = mybir.dt
    f32 = dt.float32
    f32r = dt.float32r
    AF = mybir.ActivationFunctionType

    nc = bacc.Bacc("TRN2", target_bir_lowering=False, debug=False)

    image = nc.dram_tensor("image", [C, T1, HW], f32, kind="ExternalInput").ap()
    motion = nc.dram_tensor("motion", [C, T2 * V], f32, kind="ExternalInput").ap()
    w_iv = nc.dram_tensor("w_iv", [C, C], f32, kind="ExternalInput").ap()
    b_iv = nc.dram_tensor("b_iv", [C], f32, kind="ExternalInput").ap()
    w_iq = nc.dram_tensor("w_iq", [REL, C], f32, kind="ExternalInput").ap()
    b_iq = nc.dram_tensor("b_iq", [REL], f32, kind="ExternalInput").ap()
    w_mq = nc.dram_tensor("w_mq", [REL, C], f32, kind="ExternalInput").ap()
    b_mq = nc.dram_tensor("b_mq", [REL], f32, kind="ExternalInput").ap()
    w_att = nc.dram_tensor("w_att", [C, REL], f32, kind="ExternalInput").ap()
    b_att = nc.dram_tensor("b_att", [C], f32, kind="ExternalInput").ap()
    y = nc.dram_tensor("y", [C, T1, HW], f32, kind="ExternalOutput").ap()

    with tile.TileContext(nc) as tc, ExitStack() as ctx:
        const = ctx.enter_context(tc.tile_pool(name="const", bufs=1))
        img_pool = ctx.enter_context(tc.tile_pool(name="imgp", bufs=4))
        stg_pool = ctx.enter_context(tc.tile_pool(name="stgp", bufs=4))
        y_pool = ctx.enter_context(tc.tile_pool(name="yp", bufs=3))
        ps_pool = ctx.enter_context(tc.tile_pool(name="ps", bufs=3, space="PSUM"))
        ps_small = ctx.enter_context(tc.tile_pool(name="pss", bufs=2, space="PSUM"))

        # ---------------- setup: constants ----------------
        ident = const.tile([128, 128], f32, name="ident", tag="ident")
        masks.make_identity(nc, ident[:])

        eye = const.tile([128, 256], f32r, name="eye", tag="eye")
        nc.vector.memset(eye[:].bitcast(f32), 0.0)
        for j in range(T1):
            nc.vector.memset(eye[:, j * 17 : j * 17 + 1].bitcast(f32), 1.0)

        # biases
        b_iq_sb = const.tile([REL, 1], f32, name="b_iq_sb", tag="b_iq_sb")
        nc.sync.dma_start(b_iq_sb[:], b_iq[:])
        b_mq_sb = const.tile([REL, 1], f32, name="b_mq_sb", tag="b_mq_sb")
        nc.sync.dma_start(b_mq_sb[:], b_mq[:])
        b_iv_sb = const.tile([128, 2], f32, name="b_iv_sb", tag="b_iv_sb")
        b_att_sb = const.tile([128, 2], f32, name="b_att_sb", tag="b_att_sb")
        for h in range(2):
            nc.sync.dma_start(b_iv_sb[:, h : h + 1], b_iv[h * CH : (h + 1) * CH])
            nc.sync.dma_start(b_att_sb[:, h : h + 1], b_att[h * CH : (h + 1) * CH])
        nc.scalar.mul(b_att_sb[:], b_att_sb[:], 1.0 / T1)

        # ---------------- setup: weight transposes ----------------
        # w_iv -> lhsT_iv blocks [c'(128), d(128)] at cols (kh*2+h2)*128
        w_iv_sb = const.tile([128, 512], f32, name="w_iv_sb", tag="w_iv_sb")
        for h in range(2):
            nc.sync.dma_start(
                w_iv_sb[:, h * 256 : (h + 1) * 256], w_iv[h * CH : (h + 1) * CH, :]
            )
        lhsT_iv = const.tile([128, 512], f32r, name="lhsT_iv", tag="lhsT_iv")
        for kh in range(2):
            for h2 in range(2):
                tp = ps_small.tile([128, 256], f32, name="tp", tag="pss")
                nc.tensor.transpose(
                    tp[:, 0:128],
                    w_iv_sb[:, h2 * 256 + kh * 128 : h2 * 256 + (kh + 1) * 128],
                    ident[:],
                )
                nc.scalar.copy(
                    lhsT_iv[:, (kh * 2 + h2) * 128 : (kh * 2 + h2 + 1) * 128],
                    tp[:, 0:128],
                )

        # w_iq / w_mq -> lhsT [c'(128), r(32)] at cols kh*32, scaled by 1/HW, 1/100
        w_iq_sb = const.tile([REL, C], f32, name="w_iq_sb", tag="w_iq_sb")
        nc.sync.dma_start(w_iq_sb[:], w_iq[:])
        w_mq_sb = const.tile([REL, C], f32, name="w_mq_sb", tag="w_mq_sb")
        nc.sync.dma_start(w_mq_sb[:], w_mq[:])
        lhsT_iq = const.tile([128, 64], f32r, name="lhsT_iq", tag="lhsT_iq")
        lhsT_mq = const.tile([128, 64], f32r, name="lhsT_mq", tag="lhsT_mq")
        for kh in range(2):
            tp = ps_small.tile([128, 256], f32, name="tp", tag="pss")
            nc.tensor.transpose(
                tp[:, 0:32], w_iq_sb[:, kh * 128 : (kh + 1) * 128], ident[0:32, 0:32]
            )
            nc.scalar.mul(lhsT_iq[:, kh * 32 : (kh + 1) * 32], tp[:, 0:32], 1.0 / HW)
            tp2 = ps_small.tile([128, 256], f32, name="tp2", tag="pss")
            nc.tensor.transpose(
                tp2[:, 0:32], w_mq_sb[:, kh * 128 : (kh + 1) * 128], ident[0:32, 0:32]
            )
            nc.scalar.mul(
                lhsT_mq[:, kh * 32 : (kh + 1) * 32],
                tp2[:, 0:32],
                1.0 / ((T2 // T1) * V),
            )

        # w_att [C, REL] -> lhsT_att [r(32), c(256)]
        w_att_sb = const.tile([128, 64], f32, name="w_att_sb", tag="w_att_sb")
        for h in range(2):
            nc.sync.dma_start(
                w_att_sb[:, h * 32 : (h + 1) * 32], w_att[h * CH : (h + 1) * CH, :]
            )
        lhsT_att = const.tile([REL, 256], f32r, name="lhsT_att", tag="lhsT_att")
        for h in range(2):
            tp = ps_small.tile([128, 256], f32, name="tp", tag="pss")
            nc.tensor.transpose(
                tp[0:32, 0:128], w_att_sb[:, h * 32 : (h + 1) * 32], ident[:]
            )
            nc.scalar.copy(lhsT_att[:, h * 128 : (h + 1) * 128], tp[0:32, 0:128])

        # block-diagonal att buffer, zeroed once
        lhsT_bd = const.tile([128, NG * 128], f32r, name="lhsT_bd", tag="lhsT_bd")
        nc.gpsimd.memset(lhsT_bd[:].bitcast(f32), 0.0)

        # ---------------- motion pooling ----------------
        mot_pool = const.tile([128, 32], f32r, name="mot_pool", tag="mot_pool")
        for h in range(2):
            mot = const.tile([128, T2 * V], f32, name="mot", tag=f"mot{h}")
            nc.sync.dma_start(mot[:], motion[h * CH : (h + 1) * CH, :])
            with nc.allow_low_precision(reason="f32r output, fp32 accumulate"):
                nc.vector.reduce_sum(
                    mot_pool[:, h * T1 : (h + 1) * T1],
                    mot[:].rearrange("p (t q) -> p t q", q=(T2 // T1) * V),
                    axis=mybir.AxisListType.X,
                )

        # ---------------- phase 1: stream image ----------------
        V_fold = const.tile([128, NG * HW], f32r, name="V_fold", tag="V_fold")
        img_sum = const.tile([128, 32], f32r, name="img_sum", tag="img_sum")

        for t in range(T1):
            imgs = []
            for h in range(2):
                img = img_pool.tile([128, HW], f32r, name="img", tag="img")
                nc.sync.dma_start(
                    img[:], image[h * CH : (h + 1) * CH, t, :].bitcast(f32r)
                )
                imgs.append(img)
                # per-(c,t) spatial sum for Q
                with nc.allow_low_precision(reason="f32r output, fp32 accumulate"):
                    nc.vector.reduce_sum(
                        img_sum[:, h * T1 + t : h * T1 + t + 1],
                        img[:].bitcast(f32),
                        axis=mybir.AxisListType.X,
                    )
            for h2 in range(2):
                vv = ps_pool.tile([128, 1024], f32, name="vv", tag="mm")
                for kh in range(2):
                    for cs, pcol in ((0, 0), (392, 512)):
                        nc.tensor.matmul(
                            vv[:, pcol : pcol + 392],
                            lhsT_iv[:, (kh * 2 + h2) * 128 : (kh * 2 + h2 + 1) * 128],
                            imgs[kh][:, cs : cs + 392],
                            start=(kh == 0),
                            stop=(kh == 1),
                        )
                stg = stg_pool.tile([128, HW], f32r, name="stg", tag="stg")
                nc.scalar.activation(
                    stg[:].rearrange("p (c x) -> p c x", x=392),
                    vv[:].rearrange("p (c x) -> p c x", x=512)[:, :, 0:392],
                    AF.Identity,
                    bias=b_iv_sb[:, h2 : h2 + 1],
                )
                # fold: V_fold[16*(4*h2+cl)+t, g*HW+v] = stg[32*cl+g, v]
                # one DMA per (t, h2): src [128, 784] -> dst [4 partitions
                # (step 16), 25088 contiguous]
                nc.scalar.dma_start(
                    V_fold[:].rearrange("(cl r) q -> cl r q", r=T1)[
                        4 * h2 : 4 * h2 + 4, t
                    ],
                    stg[:],
                )

        # ---------------- phase 2: attention ----------------
        q_ps = ps_small.tile([128, 256], f32, name="q_ps", tag="pss")
        k_ps = ps_small.tile([128, 256], f32, name="k_ps", tag="pss")
        for kh in range(2):
            nc.tensor.matmul(
                q_ps[0:REL, 0:T1],
                lhsT_iq[:, kh * 32 : (kh + 1) * 32],
                img_sum[:, kh * T1 : (kh + 1) * T1],
                start=(kh == 0),
                stop=(kh == 1),
            )
            nc.tensor.matmul(
                k_ps[0:REL, 0:T1],
                lhsT_mq[:, kh * 32 : (kh + 1) * 32],
                mot_pool[:, kh * T1 : (kh + 1) * T1],
                start=(kh == 0),
                stop=(kh == 1),
            )
        q_sb = const.tile([REL, T1], f32, name="q_sb", tag="q_sb")
        nc.scalar.activation(
            q_sb[:], q_ps[0:REL, 0:T1], AF.Identity, bias=b_iq_sb[:, 0:1]
        )
        k_sb = const.tile([REL, T1], f32, name="k_sb", tag="k_sb")
        nc.scalar.activation(
            k_sb[:], k_ps[0:REL, 0:T1], AF.Identity, bias=b_mq_sb[:, 0:1]
        )

        # s2[r, j*16+i] = tanh(K[r,i] - Q[r,j])
        diff = const.tile([REL, 256], f32, name="diff", tag="diff")
        nc.vector.tensor_sub(
            diff[:].rearrange("p (j i) -> p j i", i=T1),
            k_sb[:].unsqueeze(1).broadcast_to((REL, T1, T1)),
            q_sb[:].unsqueeze(2).broadcast_to((REL, T1, T1)),
        )
        s2 = const.tile([REL, 256], f32r, name="s2", tag="s2")
        nc.scalar.activation(s2[:], diff[:], AF.Tanh)

        # att_sb_h[c_loc, j*16+i] = att[c, i, j] = (w_att@s2 + b_att)/16 + I
        att_sbs = []
        for h in range(2):
            a_ps = ps_small.tile([128, 256], f32, name="a_ps", tag="pss")
            nc.tensor.matmul(
                a_ps[:, 0:256],
                lhsT_att[:, h * 128 : (h + 1) * 128],
                s2[:],
            )
            att_sb = const.tile([128, 256], f32r, name="att_sb", tag=f"att_sb{h}")
            nc.scalar.activation(
                att_sb[:],
                a_ps[:, 0:256],
                AF.Identity,
                scale=1.0 / T1,
                bias=b_att_sb[:, h : h + 1],
            )
            nc.vector.tensor_add(att_sb[:], att_sb[:], eye[:])
            att_sbs.append(att_sb)

        # scatter into block-diagonal lhsT:
        # lhsT_bd[16*cl+j, g*128+16*cl+i] = att_sb[32*cl+g, j*16+i]
        # dst col offset 16*cl is partition-dependent (block diagonal), so the
        # scatter cannot coalesce across cl: per-(cl, j) DMAs, alternating
        # between the two HWDGE rings.
        for h in range(2):
            for j in range(T1):
                for cl_loc in range(4):
                    cl = 4 * h + cl_loc
                    s = att_sbs[h][
                        32 * cl_loc : 32 * cl_loc + 32, j * T1 : (j + 1) * T1
                    ]
                    d = lhsT_bd[16 * cl + j : 16 * cl + j + 1, :].rearrange(
                        "p (g c) -> p g c", c=128
                    )[:, :, 16 * cl : 16 * cl + 16]
                    eng = nc.scalar if (j + cl_loc) % 2 == 0 else nc.sync
                    eng.dma_start(d, s)

        # ---------------- phase 3: y = att_bd.T @ V_fold ----------------
        for g in range(NG):
            yp = ps_pool.tile([128, 1024], f32, name="yp", tag="mm")
            for cs, pcol in ((0, 0), (392, 512)):
                nc.tensor.matmul(
                    yp[:, pcol : pcol + 392],
                    lhsT_bd[:, g * 128 : (g + 1) * 128],
                    V_fold[:, g * HW + cs : g * HW + cs + 392],
                )
            y_sb = y_pool.tile([128, HW], f32, name="y_sb", tag="y_sb")
            copy_eng = nc.scalar if (g % 8) < 5 else nc.vector
            if copy_eng is nc.scalar:
                nc.scalar.copy(
                    y_sb[:].rearrange("p (c x) -> p c x", x=392),
                    yp[:].rearrange("p (c x) -> p c x", x=512)[:, :, 0:392],
                )
            else:
                nc.vector.tensor_copy(
                    y_sb[:].rearrange("p (c x) -> p c x", x=392),
                    yp[:].rearrange("p (c x) -> p c x", x=512)[:, :, 0:392],
                )
            out_eng = nc.sync if g % 2 == 0 else nc.scalar
            out_eng.dma_start(
                y[:].rearrange("(cl g) t v -> cl g t v", g=NG)[:, g],
                y_sb[:],
            )

    nc.compile()
    return nc


def _get_nc():
    if "nc" not in _CACHE:
        _CACHE["nc"] = _build()
    return _CACHE["nc"]


def kernel(**inputs) -> np.ndarray:
    from concourse.bass_utils import run_bass_kernel_spmd

    nc = _get_nc()

    image = np.ascontiguousarray(np.asarray(inputs["image"], dtype=np.float32))
    motion = np.ascontiguousarray(np.asarray(inputs["motion"], dtype=np.float32))
    shared = {
        k: np.ascontiguousarray(np.asarray(inputs[k], dtype=np.float32))
        for k in ("w_iv", "b_iv", "w_iq", "b_iq", "w_mq", "b_mq", "w_att", "b_att")
    }
    in_maps = []
    for n in range(NCORES):
        m = {
            "image": image[n].reshape(C, T1, HW),
            "motion": motion[n].reshape(C, T2 * V),
        }
        m.update(shared)
        in_maps.append(m)

    res = run_bass_kernel_spmd(nc, in_maps, core_ids=list(range(NCORES)))
    out = np.stack([res.results[n]["y"] for n in range(NCORES)], axis=0)
    return out.reshape(N, C, T1, H, W)



# revision 6
# speedup vs baseline: 1.2637x; 1.2637x over previous
"""Trainium2 Bass kernel for CrossModalAttention.

Reference computation (per sample n, data-parallel over 8 cores):
  img_mean[c,t]  = mean_v image[c,t,v]
  Q[r,t]         = w_iq @ img_mean + b_iq
  K[r,t]         = w_mq @ (block-mean of motion) + b_mq
  Vv[c,t,v]      = w_iv @ image + b_iv
  s[r,i,j]       = tanh(K[r,i] - Q[r,j])
  att[c,i,j]     = (w_att @ s + b_att)/T1 + I
  y[c,i,v]       = sum_j att[c,i,j] * Vv[c,j,v]

v2 data-movement design (v1 was DMA-bound at 409us: all traffic on the two
HWDGE rings, fold+input+output serialized there):
 - image loads: one DMA per 2-t pair (6272B per-partition chunks),
   alternating sync/scalar HWDGE rings -> each ring carries 6.4MB.
 - Vv is evicted PSUM->SBUF in bf16 (ACT casts, fused bias), halving the
   SBUF->SBUF fold traffic; fold DMAs go on the gpsimd SWDGE queue (3rd
   queue row), one per (t, h2).
 - attention chain: K-side emitted early (overlaps phase 1), att matrices
   cast to bf16; block-diag scatter round-robins all 3 queue rows.
 - y matmuls in bf16 (lhsT_bd bf16 x V_fold bf16), PSUM f32; evicts
   alternate scalar/vector; output DMAs alternate sync/scalar rings.
"""

import numpy as np

N, C, T1, H, W = 8, 256, 16, 28, 28
HW = H * W
T2, V = 64, 25
REL = 32
CH = 128  # partition half of C
NCORES = 8
NG = 32  # channel groups (c = 32*cl + g)

_CACHE = {}


def _build():
    from contextlib import ExitStack
    from concourse import bass, mybir, tile, bacc, masks

    dt = mybir.dt
    f32 = dt.float32
    f32r = dt.float32r
    bf16 = dt.bfloat16
    AF = mybir.ActivationFunctionType

    nc = bacc.Bacc("TRN2", target_bir_lowering=False, debug=False)

    image = nc.dram_tensor("image", [C, T1, HW], f32, kind="ExternalInput").ap()
    motion = nc.dram_tensor("motion", [C, T2 * V], f32, kind="ExternalInput").ap()
    w_iv = nc.dram_tensor("w_iv", [C, C], f32, kind="ExternalInput").ap()
    b_iv = nc.dram_tensor("b_iv", [C], f32, kind="ExternalInput").ap()
    w_iq = nc.dram_tensor("w_iq", [REL, C], f32, kind="ExternalInput").ap()
    b_iq = nc.dram_tensor("b_iq", [REL], f32, kind="ExternalInput").ap()
    w_mq = nc.dram_tensor("w_mq", [REL, C], f32, kind="ExternalInput").ap()
    b_mq = nc.dram_tensor("b_mq", [REL], f32, kind="ExternalInput").ap()
    w_att = nc.dram_tensor("w_att", [C, REL], f32, kind="ExternalInput").ap()
    b_att = nc.dram_tensor("b_att", [C], f32, kind="ExternalInput").ap()
    y = nc.dram_tensor("y", [C, T1, HW], f32, kind="ExternalOutput").ap()

    with tile.TileContext(nc) as tc, ExitStack() as ctx:
        const = ctx.enter_context(tc.tile_pool(name="const", bufs=1))
        img_pool = ctx.enter_context(tc.tile_pool(name="imgp", bufs=2))
        stg_pool = ctx.enter_context(tc.tile_pool(name="stgp", bufs=4))
        y_pool = ctx.enter_context(tc.tile_pool(name="yp", bufs=3))
        ps_pool = ctx.enter_context(tc.tile_pool(name="ps", bufs=3, space="PSUM"))
        ps_small = ctx.enter_context(tc.tile_pool(name="pss", bufs=2, space="PSUM"))

        # ---------------- setup: constants ----------------
        ident = const.tile([128, 128], f32, name="ident", tag="ident")
        masks.make_identity(nc, ident[:])

        eye = const.tile([128, 256], f32, name="eye", tag="eye")
        nc.vector.memset(eye[:], 0.0)
        for j in range(T1):
            nc.vector.memset(eye[:, j * 17 : j * 17 + 1], 1.0)

        # biases
        b_iq_sb = const.tile([REL, 1], f32, name="b_iq_sb", tag="b_iq_sb")
        nc.sync.dma_start(b_iq_sb[:], b_iq[:])
        b_mq_sb = const.tile([REL, 1], f32, name="b_mq_sb", tag="b_mq_sb")
        nc.sync.dma_start(b_mq_sb[:], b_mq[:])
        b_iv_sb = const.tile([128, 2], f32, name="b_iv_sb", tag="b_iv_sb")
        b_att_sb = const.tile([128, 2], f32, name="b_att_sb", tag="b_att_sb")
        for h in range(2):
            nc.sync.dma_start(b_iv_sb[:, h : h + 1], b_iv[h * CH : (h + 1) * CH])
            nc.sync.dma_start(b_att_sb[:, h : h + 1], b_att[h * CH : (h + 1) * CH])
        nc.scalar.mul(b_att_sb[:], b_att_sb[:], 1.0 / T1)

        # ---------------- setup: weight transposes ----------------
        # w_iv -> lhsT_iv blocks [c'(128), d(128)] at cols (kh*2+h2)*128
        w_iv_sb = const.tile([128, 512], f32, name="w_iv_sb", tag="w_iv_sb")
        for h in range(2):
            nc.sync.dma_start(
                w_iv_sb[:, h * 256 : (h + 1) * 256], w_iv[h * CH : (h + 1) * CH, :]
            )
        lhsT_iv = const.tile([128, 512], f32r, name="lhsT_iv", tag="lhsT_iv")
        for kh in range(2):
            for h2 in range(2):
                tp = ps_small.tile([128, 256], f32, name="tp", tag="pss")
                nc.tensor.transpose(
                    tp[:, 0:128],
                    w_iv_sb[:, h2 * 256 + kh * 128 : h2 * 256 + (kh + 1) * 128],
                    ident[:],
                )
                nc.scalar.copy(
                    lhsT_iv[:, (kh * 2 + h2) * 128 : (kh * 2 + h2 + 1) * 128],
                    tp[:, 0:128],
                )

        # w_iq / w_mq -> lhsT [c'(128), r(32)] at cols kh*32, scaled by 1/HW, 1/100
        w_iq_sb = const.tile([REL, C], f32, name="w_iq_sb", tag="w_iq_sb")
        nc.sync.dma_start(w_iq_sb[:], w_iq[:])
        w_mq_sb = const.tile([REL, C], f32, name="w_mq_sb", tag="w_mq_sb")
        nc.sync.dma_start(w_mq_sb[:], w_mq[:])
        lhsT_iq = const.tile([128, 64], f32r, name="lhsT_iq", tag="lhsT_iq")
        lhsT_mq = const.tile([128, 64], f32r, name="lhsT_mq", tag="lhsT_mq")
        for kh in range(2):
            tp = ps_small.tile([128, 256], f32, name="tp", tag="pss")
            nc.tensor.transpose(
                tp[:, 0:32], w_iq_sb[:, kh * 128 : (kh + 1) * 128], ident[0:32, 0:32]
            )
            nc.scalar.mul(lhsT_iq[:, kh * 32 : (kh + 1) * 32], tp[:, 0:32], 1.0 / HW)
            tp2 = ps_small.tile([128, 256], f32, name="tp2", tag="pss")
            nc.tensor.transpose(
                tp2[:, 0:32], w_mq_sb[:, kh * 128 : (kh + 1) * 128], ident[0:32, 0:32]
            )
            nc.scalar.mul(
                lhsT_mq[:, kh * 32 : (kh + 1) * 32],
                tp2[:, 0:32],
                1.0 / ((T2 // T1) * V),
            )

        # w_att [C, REL] -> lhsT_att [r(32), c(256)]
        w_att_sb = const.tile([128, 64], f32, name="w_att_sb", tag="w_att_sb")
        for h in range(2):
            nc.sync.dma_start(
                w_att_sb[:, h * 32 : (h + 1) * 32], w_att[h * CH : (h + 1) * CH, :]
            )
        lhsT_att = const.tile([REL, 256], f32r, name="lhsT_att", tag="lhsT_att")
        for h in range(2):
            tp = ps_small.tile([128, 256], f32, name="tp", tag="pss")
            nc.tensor.transpose(
                tp[0:32, 0:128], w_att_sb[:, h * 32 : (h + 1) * 32], ident[:]
            )
            nc.scalar.copy(lhsT_att[:, h * 128 : (h + 1) * 128], tp[0:32, 0:128])

        # block-diagonal att buffer (bf16), zeroed once
        lhsT_bd = const.tile([128, NG * 128], bf16, name="lhsT_bd", tag="lhsT_bd")
        nc.gpsimd.memset(lhsT_bd[:].bitcast(f32), 0.0)

        # ---------------- motion pooling ----------------
        mot_pool = const.tile([128, 32], f32r, name="mot_pool", tag="mot_pool")
        for h in range(2):
            mot = const.tile([128, T2 * V], f32, name="mot", tag=f"mot{h}")
            nc.gpsimd.dma_start(mot[:], motion[h * CH : (h + 1) * CH, :])
            with nc.allow_low_precision(reason="f32r output, fp32 accumulate"):
                nc.vector.reduce_sum(
                    mot_pool[:, h * T1 : (h + 1) * T1],
                    mot[:].rearrange("p (t q) -> p t q", q=(T2 // T1) * V),
                    axis=mybir.AxisListType.X,
                )

        # ---------------- phase 1: stream image ----------------
        # V_fold[16*cl+j, g*HW+v] = Vv[c,j,v] (bf16), c = 32*cl+g, cl = 4*h2+q
        V_fold = const.tile([128, NG * HW], bf16, name="V_fold", tag="V_fold")
        img_sum = const.tile([128, 32], f32r, name="img_sum", tag="img_sum")

        k_sb = const.tile([REL, T1], f32, name="k_sb", tag="k_sb")

        for tp_i in range(T1 // 2):
            # one load per 2 t's: img[p, h*2*HW + tt*HW + v] =
            # image[h*128+p, 2*tp_i+tt, v] -- per-(p,h) 6272B contiguous both
            # sides, so the DMA moves 6272B packets.
            img = img_pool.tile([128, 2 * 2 * HW], f32r, name="img", tag="img")
            eng = nc.sync if tp_i % 2 == 0 else nc.scalar
            eng.dma_start(
                img[:].rearrange("p (h tt v) -> p h tt v", h=2, v=HW),
                image[:, 2 * tp_i : 2 * tp_i + 2, :]
                .rearrange("(h p) tt v -> p h tt v", h=2)
                .bitcast(f32r),
            )
            imgv = img[:].rearrange("p (h tt v) -> p h tt v", h=2, v=HW)
            for tt in range(2):
                t = 2 * tp_i + tt
                # per-(c,t) spatial sum for Q: img_sum[:, h*16+t]
                for h in range(2):
                    with nc.allow_low_precision(reason="f32r out, fp32 accumulate"):
                        nc.vector.reduce_sum(
                            img_sum[:, h * T1 + t : h * T1 + t + 1],
                            imgv[:, h, tt].bitcast(f32),
                            axis=mybir.AxisListType.X,
                        )
                stg = stg_pool.tile([128, 2 * HW], bf16, name="stg", tag="stg")
                for h2 in range(2):
                    vv = ps_pool.tile([128, 1024], f32, name="vv", tag="mm")
                    for kh in range(2):
                        for cs, pcol in ((0, 0), (392, 512)):
                            nc.tensor.matmul(
                                vv[:, pcol : pcol + 392],
                                lhsT_iv[
                                    :, (kh * 2 + h2) * 128 : (kh * 2 + h2 + 1) * 128
                                ],
                                imgv[:, kh, tt, cs : cs + 392],
                                start=(kh == 0),
                                stop=(kh == 1),
                            )
                    nc.scalar.activation(
                        stg[:, h2 * HW : (h2 + 1) * HW].rearrange(
                            "p (c x) -> p c x", x=392
                        ),
                        vv[:].rearrange("p (c x) -> p c x", x=512)[:, :, 0:392],
                        AF.Identity,
                        bias=b_iv_sb[:, h2 : h2 + 1],
                    )
                # fold (gpsimd SWDGE): per (t, h2):
                # V_fold[16*(4*h2+q)+t, g*HW+v] = stg[32*q+g, h2*HW+v]
                for h2 in range(2):
                    nc.gpsimd.dma_start(
                        V_fold[:]
                        .rearrange("(h q r) (g v) -> h q r g v", h=2, q=4, v=HW)[
                            h2, :, t
                        ],
                        stg[:, h2 * HW : (h2 + 1) * HW].rearrange(
                            "(q g) v -> q g v", q=4
                        ),
                    )

            if tp_i == 1:
                # K-side of attention: depends only on motion; emit early so it
                # overlaps phase 1 instead of sitting in the phase boundary.
                k_ps = ps_small.tile([128, 256], f32, name="k_ps", tag="pss")
                for kh in range(2):
                    nc.tensor.matmul(
                        k_ps[0:REL, 0:T1],
                        lhsT_mq[:, kh * 32 : (kh + 1) * 32],
                        mot_pool[:, kh * T1 : (kh + 1) * T1],
                        start=(kh == 0),
                        stop=(kh == 1),
                    )
                nc.scalar.activation(
                    k_sb[:], k_ps[0:REL, 0:T1], AF.Identity, bias=b_mq_sb[:, 0:1]
                )

        # ---------------- phase 2: attention ----------------
        q_ps = ps_small.tile([128, 256], f32, name="q_ps", tag="pss")
        for kh in range(2):
            nc.tensor.matmul(
                q_ps[0:REL, 0:T1],
                lhsT_iq[:, kh * 32 : (kh + 1) * 32],
                img_sum[:, kh * T1 : (kh + 1) * T1],
                start=(kh == 0),
                stop=(kh == 1),
            )
        q_sb = const.tile([REL, T1], f32, name="q_sb", tag="q_sb")
        nc.scalar.activation(
            q_sb[:], q_ps[0:REL, 0:T1], AF.Identity, bias=b_iq_sb[:, 0:1]
        )

        # s2[r, j*16+i] = tanh(K[r,i] - Q[r,j])
        diff = const.tile([REL, 256], f32, name="diff", tag="diff")
        nc.vector.tensor_sub(
            diff[:].rearrange("p (j i) -> p j i", i=T1),
            k_sb[:].unsqueeze(1).broadcast_to((REL, T1, T1)),
            q_sb[:].unsqueeze(2).broadcast_to((REL, T1, T1)),
        )
        s2 = const.tile([REL, 256], f32r, name="s2", tag="s2")
        nc.scalar.activation(s2[:], diff[:], AF.Tanh)

        # att_sb_h[c_loc, j*16+i] = att[c, i, j] = (w_att@s2 + b_att)/16 + I
        att_sbs = []
        for h in range(2):
            a_ps = ps_small.tile([128, 256], f32, name="a_ps", tag="pss")
            nc.tensor.matmul(
                a_ps[:, 0:256],
                lhsT_att[:, h * 128 : (h + 1) * 128],
                s2[:],
            )
            att_f = const.tile([128, 256], f32, name="att_f", tag=f"att_f{h}")
            nc.scalar.activation(
                att_f[:],
                a_ps[:, 0:256],
                AF.Identity,
                scale=1.0 / T1,
                bias=b_att_sb[:, h : h + 1],
            )
            att_sb = const.tile([128, 256], bf16, name="att_sb", tag=f"att_sb{h}")
            nc.vector.tensor_add(att_sb[:], att_f[:], eye[:])
            att_sbs.append(att_sb)

        # scatter into block-diagonal lhsT (bf16):
        # lhsT_bd[16*cl+j, g*128+16*cl+i] = att_sb[32*cl_loc+g, j*16+i]
        engs = [nc.sync, nc.scalar, nc.gpsimd]
        di = 0
        for h in range(2):
            for j in range(T1):
                for cl_loc in range(4):
                    cl = 4 * h + cl_loc
                    s = att_sbs[h][
                        32 * cl_loc : 32 * cl_loc + 32, j * T1 : (j + 1) * T1
                    ]
                    d = lhsT_bd[16 * cl + j : 16 * cl + j + 1, :].rearrange(
                        "p (g c) -> p g c", c=128
                    )[:, :, 16 * cl : 16 * cl + 16]
                    engs[di % 3].dma_start(d, s)
                    di += 1

        # ---------------- phase 3: y = att_bd.T @ V_fold ----------------
        for g in range(NG):
            yp = ps_pool.tile([128, 1024], f32, name="yp", tag="mm")
            for cs, pcol in ((0, 0), (392, 512)):
                nc.tensor.matmul(
                    yp[:, pcol : pcol + 392],
                    lhsT_bd[:, g * 128 : (g + 1) * 128],
                    V_fold[:, g * HW + cs : g * HW + cs + 392],
                )
            y_sb = y_pool.tile([128, HW], f32, name="y_sb", tag="y_sb")
            copy_eng = nc.scalar if g % 2 == 0 else nc.vector
            if copy_eng is nc.scalar:
                nc.scalar.copy(
                    y_sb[:].rearrange("p (c x) -> p c x", x=392),
                    yp[:].rearrange("p (c x) -> p c x", x=512)[:, :, 0:392],
                )
            else:
                nc.vector.tensor_copy(
                    y_sb[:].rearrange("p (c x) -> p c x", x=392),
                    yp[:].rearrange("p (c x) -> p c x", x=512)[:, :, 0:392],
                )
            out_eng = nc.sync if g % 2 == 0 else nc.scalar
            out_eng.dma_start(
                y[:].rearrange("(cl g) t v -> cl g t v", g=NG)[:, g],
                y_sb[:],
            )

    nc.compile()
    return nc


def _get_nc():
    if "nc" not in _CACHE:
        _CACHE["nc"] = _build()
    return _CACHE["nc"]


def kernel(**inputs) -> np.ndarray:
    from concourse.bass_utils import run_bass_kernel_spmd

    nc = _get_nc()

    image = np.ascontiguousarray(np.asarray(inputs["image"], dtype=np.float32))
    motion = np.ascontiguousarray(np.asarray(inputs["motion"], dtype=np.float32))
    shared = {
        k: np.ascontiguousarray(np.asarray(inputs[k], dtype=np.float32))
        for k in ("w_iv", "b_iv", "w_iq", "b_iq", "w_mq", "b_mq", "w_att", "b_att")
    }
    in_maps = []
    for n in range(NCORES):
        m = {
            "image": image[n].reshape(C, T1, HW),
            "motion": motion[n].reshape(C, T2 * V),
        }
        m.update(shared)
        in_maps.append(m)

    res = run_bass_kernel_spmd(nc, in_maps, core_ids=list(range(NCORES)))
    out = np.stack([res.results[n]["y"] for n in range(NCORES)], axis=0)
    return out.reshape(N, C, T1, H, W)


# revision 10
# speedup vs baseline: 1.6583x; 1.3123x over previous
"""Trainium2 Bass kernel for CrossModalAttention.

Reference computation (per sample n, data-parallel over 8 cores):
  img_mean[c,t]  = mean_v image[c,t,v]
  Q[r,t]         = w_iq @ img_mean + b_iq
  K[r,t]         = w_mq @ (block-mean of motion) + b_mq
  Vv[c,t,v]      = w_iv @ image + b_iv
  s[r,i,j]       = tanh(K[r,i] - Q[r,j])
  att[c,i,j]     = (w_att @ s + b_att)/T1 + I
  y[c,i,v]       = sum_j att[c,i,j] * Vv[c,j,v]

v3 design. The y contraction is 32 block-diagonal PE matmuls over K=128 =
(j=16) x (cl=8 channels); all layouts below are chosen so every data
movement is a single well-formed DMA with >=64B contiguous packets:

  V_fold[8j+cl, g*784+v] = Vv[32cl+g, j, v]      (bf16)
  lhsT_bd[8j+cl, 256i+c] = att[c, i, j]          (bf16, block-diagonal)
  y-matmul g: out[m=8i+cl, v] with stationary cols {256i+32cl+g}

 - image loads: one DMA per 2 t's (6272B packets), alternating the two
   HWDGE rings; motion tiles reuse the image pool slots.
 - Vv evicted PSUM->SBUF in bf16 by ACT (fused bias); fold DMAs on the
   gpsimd SWDGE queue row write a contiguous 4-partition slice.
 - att chain: K-side emitted early (overlaps phase 1); att (incl. +I) is
   PE-transposed to put (j,i) on partitions, then 16 scatter DMAs (64B
   packets) build the block-diagonal stationary.
 - y evicts alternate scalar/vector; outputs batched 4 groups per DMA
   (8 DMAs, split sync/scalar/gpsimd).
"""

import numpy as np

N, C, T1, H, W = 8, 256, 16, 28, 28
HW = H * W
T2, V = 64, 25
REL = 32
CH = 128  # partition half of C
NCORES = 8
NG = 32  # channel groups (c = 32*cl + g)

_CACHE = {}


def _build():
    from contextlib import ExitStack
    from concourse import bass, mybir, tile, bacc, masks

    dt = mybir.dt
    f32 = dt.float32
    f32r = dt.float32r
    bf16 = dt.bfloat16
    AF = mybir.ActivationFunctionType

    nc = bacc.Bacc("TRN2", target_bir_lowering=False, debug=False)

    image = nc.dram_tensor("image", [C, T1, HW], f32, kind="ExternalInput").ap()
    motion = nc.dram_tensor("motion", [C, T2 * V], f32, kind="ExternalInput").ap()
    w_iv = nc.dram_tensor("w_iv", [C, C], f32, kind="ExternalInput").ap()
    b_iv = nc.dram_tensor("b_iv", [C], f32, kind="ExternalInput").ap()
    w_iq = nc.dram_tensor("w_iq", [REL, C], f32, kind="ExternalInput").ap()
    b_iq = nc.dram_tensor("b_iq", [REL], f32, kind="ExternalInput").ap()
    w_mq = nc.dram_tensor("w_mq", [REL, C], f32, kind="ExternalInput").ap()
    b_mq = nc.dram_tensor("b_mq", [REL], f32, kind="ExternalInput").ap()
    w_att = nc.dram_tensor("w_att", [C, REL], f32, kind="ExternalInput").ap()
    b_att = nc.dram_tensor("b_att", [C], f32, kind="ExternalInput").ap()
    y = nc.dram_tensor("y", [C, T1, HW], f32, kind="ExternalOutput").ap()

    with tile.TileContext(nc) as tc, ExitStack() as ctx:
        const = ctx.enter_context(tc.tile_pool(name="const", bufs=1))
        img_pool = ctx.enter_context(tc.tile_pool(name="imgp", bufs=3))
        stg_pool = ctx.enter_context(tc.tile_pool(name="stgp", bufs=5))
        y_pool = ctx.enter_context(tc.tile_pool(name="yp", bufs=4))
        ps_pool = ctx.enter_context(tc.tile_pool(name="ps", bufs=3, space="PSUM"))
        ps_small = ctx.enter_context(tc.tile_pool(name="pss", bufs=2, space="PSUM"))

        # ---------------- setup: constants ----------------
        ident = const.tile([128, 128], f32, name="ident", tag="ident")
        masks.make_identity(nc, ident[:])

        eye = const.tile([128, 256], f32, name="eye", tag="eye")
        nc.vector.memset(eye[:], 0.0)
        for j in range(T1):
            nc.vector.memset(eye[:, j * 17 : j * 17 + 1], 1.0)

        # biases
        b_iq_sb = const.tile([REL, 1], f32, name="b_iq_sb", tag="b_iq_sb")
        nc.sync.dma_start(b_iq_sb[:], b_iq[:])
        b_mq_sb = const.tile([REL, 1], f32, name="b_mq_sb", tag="b_mq_sb")
        nc.sync.dma_start(b_mq_sb[:], b_mq[:])
        b_iv_sb = const.tile([128, 2], f32, name="b_iv_sb", tag="b_iv_sb")
        b_att_sb = const.tile([128, 2], f32, name="b_att_sb", tag="b_att_sb")
        for h in range(2):
            nc.sync.dma_start(b_iv_sb[:, h : h + 1], b_iv[h * CH : (h + 1) * CH])
            nc.sync.dma_start(b_att_sb[:, h : h + 1], b_att[h * CH : (h + 1) * CH])
        nc.scalar.mul(b_att_sb[:], b_att_sb[:], 1.0 / T1)

        # ---------------- setup: weight transposes ----------------
        # w_iv -> lhsT_iv blocks [c'(128), d(128)] at cols (kh*2+h2)*128
        w_iv_sb = const.tile([128, 512], f32, name="w_iv_sb", tag="w_iv_sb")
        for h in range(2):
            nc.sync.dma_start(
                w_iv_sb[:, h * 256 : (h + 1) * 256], w_iv[h * CH : (h + 1) * CH, :]
            )
        lhsT_iv = const.tile([128, 512], f32r, name="lhsT_iv", tag="lhsT_iv")
        for kh in range(2):
            for h2 in range(2):
                tp = ps_small.tile([128, 256], f32, name="tp", tag="pss")
                nc.tensor.transpose(
                    tp[:, 0:128],
                    w_iv_sb[:, h2 * 256 + kh * 128 : h2 * 256 + (kh + 1) * 128],
                    ident[:],
                )
                nc.scalar.copy(
                    lhsT_iv[:, (kh * 2 + h2) * 128 : (kh * 2 + h2 + 1) * 128],
                    tp[:, 0:128],
                )

        # w_iq / w_mq -> lhsT [c'(128), r(32)] at cols kh*32, scaled by 1/HW, 1/100
        w_iq_sb = const.tile([REL, C], f32, name="w_iq_sb", tag="w_iq_sb")
        nc.sync.dma_start(w_iq_sb[:], w_iq[:])
        w_mq_sb = const.tile([REL, C], f32, name="w_mq_sb", tag="w_mq_sb")
        nc.sync.dma_start(w_mq_sb[:], w_mq[:])
        lhsT_iq = const.tile([128, 64], f32r, name="lhsT_iq", tag="lhsT_iq")
        lhsT_mq = const.tile([128, 64], f32r, name="lhsT_mq", tag="lhsT_mq")
        for kh in range(2):
            tp = ps_small.tile([128, 256], f32, name="tp", tag="pss")
            nc.tensor.transpose(
                tp[:, 0:32], w_iq_sb[:, kh * 128 : (kh + 1) * 128], ident[0:32, 0:32]
            )
            nc.scalar.mul(lhsT_iq[:, kh * 32 : (kh + 1) * 32], tp[:, 0:32], 1.0 / HW)
            tp2 = ps_small.tile([128, 256], f32, name="tp2", tag="pss")
            nc.tensor.transpose(
                tp2[:, 0:32], w_mq_sb[:, kh * 128 : (kh + 1) * 128], ident[0:32, 0:32]
            )
            nc.scalar.mul(
                lhsT_mq[:, kh * 32 : (kh + 1) * 32],
                tp2[:, 0:32],
                1.0 / ((T2 // T1) * V),
            )

        # w_att [C, REL] -> lhsT_att [r(32), c(256)]
        w_att_sb = const.tile([128, 64], f32, name="w_att_sb", tag="w_att_sb")
        for h in range(2):
            nc.sync.dma_start(
                w_att_sb[:, h * 32 : (h + 1) * 32], w_att[h * CH : (h + 1) * CH, :]
            )
        lhsT_att = const.tile([REL, 256], f32r, name="lhsT_att", tag="lhsT_att")
        for h in range(2):
            tp = ps_small.tile([128, 256], f32, name="tp", tag="pss")
            nc.tensor.transpose(
                tp[0:32, 0:128], w_att_sb[:, h * 32 : (h + 1) * 32], ident[:]
            )
            nc.scalar.copy(lhsT_att[:, h * 128 : (h + 1) * 128], tp[0:32, 0:128])

        # block-diagonal att buffer (bf16), zeroed once
        # lhsT_bd[8j+cl, 256i + 32cl + g] = att[32cl+g, i, j]
        lhsT_bd = const.tile([128, NG * 128], bf16, name="lhsT_bd", tag="lhsT_bd")
        nc.gpsimd.memset(lhsT_bd[:].bitcast(f32), 0.0)

        # ---------------- motion pooling (tiles share the image pool) ----
        mot_pool = const.tile([128, 32], f32r, name="mot_pool", tag="mot_pool")
        mots = []
        for h in range(2):
            mot = img_pool.tile([128, 4 * HW], f32r, name="img", tag="img")
            nc.gpsimd.dma_start(
                mot[:].bitcast(f32)[:, 0 : T2 * V], motion[h * CH : (h + 1) * CH, :]
            )
            mots.append(mot)
        for h in range(2):
            with nc.allow_low_precision(reason="f32r output, fp32 accumulate"):
                nc.vector.reduce_sum(
                    mot_pool[:, h * T1 : (h + 1) * T1],
                    mots[h][:]
                    .bitcast(f32)[:, 0 : T2 * V]
                    .rearrange("p (t q) -> p t q", q=(T2 // T1) * V),
                    axis=mybir.AxisListType.X,
                )

        # ---------------- phase 1: stream image ----------------
        # V_fold[8j+cl, g*HW+v] = Vv[c,j,v] (bf16), c = 32*cl+g, cl = 4*h2+q
        V_fold = const.tile([128, NG * HW], bf16, name="V_fold", tag="V_fold")
        img_sum = const.tile([128, 32], f32r, name="img_sum", tag="img_sum")

        k_sb = const.tile([REL, T1], f32, name="k_sb", tag="k_sb")

        for tp_i in range(T1 // 2):
            # one load per 2 t's: img[p, h*2*HW + tt*HW + v] =
            # image[h*128+p, 2*tp_i+tt, v] -- 6272B contiguous chunks.
            img = img_pool.tile([128, 2 * 2 * HW], f32r, name="img", tag="img")
            eng = nc.sync if tp_i % 2 == 0 else nc.scalar
            eng.dma_start(
                img[:].rearrange("p (h tt v) -> p h tt v", h=2, v=HW),
                image[:, 2 * tp_i : 2 * tp_i + 2, :]
                .rearrange("(h p) tt v -> p h tt v", h=2)
                .bitcast(f32r),
            )
            imgv = img[:].rearrange("p (h tt v) -> p h tt v", h=2, v=HW)
            for tt in range(2):
                t = 2 * tp_i + tt
                # per-(c,t) spatial sum for Q: img_sum[:, h*16+t]
                for h in range(2):
                    with nc.allow_low_precision(reason="f32r out, fp32 accumulate"):
                        nc.vector.reduce_sum(
                            img_sum[:, h * T1 + t : h * T1 + t + 1],
                            imgv[:, h, tt].bitcast(f32),
                            axis=mybir.AxisListType.X,
                        )
                stg = stg_pool.tile([128, 2 * HW], bf16, name="stg", tag="stg")
                for h2 in range(2):
                    vv = ps_pool.tile([128, 1024], f32, name="vv", tag="mm")
                    for kh in range(2):
                        for cs, pcol in ((0, 0), (392, 512)):
                            nc.tensor.matmul(
                                vv[:, pcol : pcol + 392],
                                lhsT_iv[
                                    :, (kh * 2 + h2) * 128 : (kh * 2 + h2 + 1) * 128
                                ],
                                imgv[:, kh, tt, cs : cs + 392],
                                start=(kh == 0),
                                stop=(kh == 1),
                            )
                    nc.scalar.activation(
                        stg[:, h2 * HW : (h2 + 1) * HW].rearrange(
                            "p (c x) -> p c x", x=392
                        ),
                        vv[:].rearrange("p (c x) -> p c x", x=512)[:, :, 0:392],
                        AF.Identity,
                        bias=b_iv_sb[:, h2 : h2 + 1],
                    )
                # fold (gpsimd SWDGE): per (t, h2):
                # V_fold[8t + 4h2 + q, g*HW+v] = stg[32q+g, h2*HW+v]
                for h2 in range(2):
                    nc.gpsimd.dma_start(
                        V_fold[8 * t + 4 * h2 : 8 * t + 4 * h2 + 4].rearrange(
                            "q (g v) -> q g v", v=HW
                        ),
                        stg[:, h2 * HW : (h2 + 1) * HW].rearrange(
                            "(q g) v -> q g v", q=4
                        ),
                    )

            if tp_i == 1:
                # K-side of attention: depends only on motion; emit early so it
                # overlaps phase 1 instead of sitting in the phase boundary.
                k_ps = ps_small.tile([128, 256], f32, name="k_ps", tag="pss")
                for kh in range(2):
                    nc.tensor.matmul(
                        k_ps[0:REL, 0:T1],
                        lhsT_mq[:, kh * 32 : (kh + 1) * 32],
                        mot_pool[:, kh * T1 : (kh + 1) * T1],
                        start=(kh == 0),
                        stop=(kh == 1),
                    )
                nc.scalar.activation(
                    k_sb[:], k_ps[0:REL, 0:T1], AF.Identity, bias=b_mq_sb[:, 0:1]
                )

        # ---------------- phase 2: attention ----------------
        q_ps = ps_small.tile([128, 256], f32, name="q_ps", tag="pss")
        for kh in range(2):
            nc.tensor.matmul(
                q_ps[0:REL, 0:T1],
                lhsT_iq[:, kh * 32 : (kh + 1) * 32],
                img_sum[:, kh * T1 : (kh + 1) * T1],
                start=(kh == 0),
                stop=(kh == 1),
            )
        q_sb = const.tile([REL, T1], f32, name="q_sb", tag="q_sb")
        nc.scalar.activation(
            q_sb[:], q_ps[0:REL, 0:T1], AF.Identity, bias=b_iq_sb[:, 0:1]
        )

        # s2[r, j*16+i] = tanh(K[r,i] - Q[r,j])
        diff = const.tile([REL, 256], f32, name="diff", tag="diff")
        nc.vector.tensor_sub(
            diff[:].rearrange("p (j i) -> p j i", i=T1),
            k_sb[:].unsqueeze(1).broadcast_to((REL, T1, T1)),
            q_sb[:].unsqueeze(2).broadcast_to((REL, T1, T1)),
        )
        s2 = const.tile([REL, 256], f32r, name="s2", tag="s2")
        nc.scalar.activation(s2[:], diff[:], AF.Tanh)

        # att_e[c_loc, j*16+i] = att[c, i, j] = (w_att@s2 + b_att)/16 + I
        att_es = []
        for h in range(2):
            a_ps = ps_small.tile([128, 256], f32, name="a_ps", tag="pss")
            nc.tensor.matmul(
                a_ps[:, 0:256],
                lhsT_att[:, h * 128 : (h + 1) * 128],
                s2[:],
            )
            att_f = const.tile([128, 256], f32, name="att_f", tag=f"att_f{h}")
            nc.scalar.activation(
                att_f[:],
                a_ps[:, 0:256],
                AF.Identity,
                scale=1.0 / T1,
                bias=b_att_sb[:, h : h + 1],
            )
            att_e = const.tile([128, 256], f32, name="att_e", tag=f"att_e{h}")
            nc.vector.tensor_add(att_e[:], att_f[:], eye[:])
            att_es.append(att_e)

        # PE-transpose att to put (j,i) on partitions:
        # attT[jh][16j'+i, c] = att[c, i, 8*jh+j']   (bf16)
        attTs = []
        for jh in range(2):
            attT = const.tile([128, 256], bf16, name="attT", tag=f"attT{jh}")
            for h in range(2):
                tps = ps_small.tile([128, 256], f32, name="tps", tag="pss")
                nc.tensor.transpose(
                    tps[:, 0:128],
                    att_es[h][:, jh * 128 : (jh + 1) * 128],
                    ident[:],
                )
                nc.scalar.copy(attT[:, h * 128 : (h + 1) * 128], tps[:, 0:128])
            attTs.append(attT)

        # scatter into the block-diagonal stationary: 16 DMAs, 64B packets.
        # lhsT_bd[8(8jh+j')+cl, 256i+32cl+g] = attT[jh][16j'+i, 32cl+g]
        bd_view = lhsT_bd[:].rearrange(
            "(j cl) (i hh clp g) -> j cl hh clp i g", j=16, hh=2, clp=4, g=32
        )
        di = 0
        for h in range(2):
            for jh in range(2):
                for cl_loc in range(4):
                    cl = 4 * h + cl_loc
                    d = bd_view[8 * jh : 8 * jh + 8, cl, h, cl_loc]
                    s = attTs[jh][:, 32 * cl : 32 * cl + 32]
                    eng = nc.sync if di % 2 == 0 else nc.scalar
                    eng.dma_start(d, s)
                    di += 1

        # ---------------- phase 3: y = att_bd.T @ V_fold ----------------
        # group g stationary: cols {256i + 32cl + g} -> out m = 8i + cl
        bd_g = lhsT_bd[:].rearrange("p (i cl g) -> p g i cl", i=16, g=32)
        out_engs = [nc.sync, nc.scalar, nc.gpsimd]
        for g in range(NG):
            yp = ps_pool.tile([128, 1024], f32, name="yp", tag="mm")
            for cs, pcol in ((0, 0), (392, 512)):
                nc.tensor.matmul(
                    yp[:, pcol : pcol + 392],
                    bd_g[:, g],
                    V_fold[:, g * HW + cs : g * HW + cs + 392],
                )
            y_sb = y_pool.tile([128, HW], f32, name="y_sb", tag="y_sb")
            copy_eng = nc.scalar if g % 2 == 0 else nc.vector
            dst = y_sb[:].rearrange("p (c x) -> p c x", x=392)
            src = yp[:].rearrange("p (c x) -> p c x", x=512)[:, :, 0:392]
            if copy_eng is nc.scalar:
                nc.scalar.copy(dst, src)
            else:
                nc.vector.tensor_copy(dst, src)
            # y[32cl + g, i, v] = y_sb[8i+cl, v]
            out_engs[g % 3].dma_start(
                y[:].rearrange("(cl g) t v -> g t cl v", g=NG)[g],
                y_sb[:],
            )

    nc.compile()
    return nc


def _get_nc():
    if "nc" not in _CACHE:
        _CACHE["nc"] = _build()
    return _CACHE["nc"]


def kernel(**inputs) -> np.ndarray:
    from concourse.bass_utils import run_bass_kernel_spmd

    nc = _get_nc()

    image = np.ascontiguousarray(np.asarray(inputs["image"], dtype=np.float32))
    motion = np.ascontiguousarray(np.asarray(inputs["motion"], dtype=np.float32))
    shared = {
        k: np.ascontiguousarray(np.asarray(inputs[k], dtype=np.float32))
        for k in ("w_iv", "b_iv", "w_iq", "b_iq", "w_mq", "b_mq", "w_att", "b_att")
    }
    in_maps = []
    for n in range(NCORES):
        m = {
            "image": image[n].reshape(C, T1, HW),
            "motion": motion[n].reshape(C, T2 * V),
        }
        m.update(shared)
        in_maps.append(m)

    res = run_bass_kernel_spmd(nc, in_maps, core_ids=list(range(NCORES)))
    out = np.stack([res.results[n]["y"] for n in range(NCORES)], axis=0)
    return out.reshape(N, C, T1, H, W)


# revision 13
# speedup vs baseline: 1.9685x; 1.1871x over previous
"""Trainium2 Bass kernel for CrossModalAttention.

Reference computation (per sample n, data-parallel over 8 cores):
  img_mean[c,t]  = mean_v image[c,t,v]
  Q[r,t]         = w_iq @ img_mean + b_iq
  K[r,t]         = w_mq @ (block-mean of motion) + b_mq
  Vv[c,t,v]      = w_iv @ image + b_iv
  s[r,i,j]       = tanh(K[r,i] - Q[r,j])
  att[c,i,j]     = (w_att @ s + b_att)/T1 + I
  y[c,i,v]       = sum_j att[c,i,j] * Vv[c,j,v]

v4 design. Error budget is 2e-2, so bulk tensors ride bf16 end to end:
image/motion are staged to DRAM as bf16 (host casts), y returns bf16
(host upcasts); all phase-1/3 matmuls are bf16 with full-width 784-col
moving operands. The y contraction is 32 block-diagonal PE matmuls over
K=128 = (j=16) x (cl=8 channels); layouts:

  V_fold[8j+cl, g*784+v] = Vv[32cl+g, j, v]      (bf16)
  lhsT_bd[8j+cl, 256i+c] = att[c, i, j]          (bf16, block-diagonal)
  y-matmul g: out[m=8i+cl, v] with stationary cols {256i+32cl+g}

All bulk DMA rides the two HWDGE rings (SWDGE descriptor emission
measured ~5x slower per byte): image loads one DMA per 4 t's (6272B
packets) alternating rings; fold (t,h2) alternates rings; 16 att
scatter DMAs (64B packets); 32 y out DMAs. gpsimd carries only motion.
"""

import numpy as np

N, C, T1, H, W = 8, 256, 16, 28, 28
HW = H * W
T2, V = 64, 25
REL = 32
CH = 128  # partition half of C
NCORES = 8
NG = 32  # channel groups (c = 32*cl + g)

_CACHE = {}


def _build():
    from contextlib import ExitStack
    from concourse import bass, mybir, tile, bacc, masks

    dt = mybir.dt
    f32 = dt.float32
    f32r = dt.float32r
    bf16 = dt.bfloat16
    AF = mybir.ActivationFunctionType

    nc = bacc.Bacc("TRN2", target_bir_lowering=False, debug=False)

    image = nc.dram_tensor("image", [C, T1, HW], bf16, kind="ExternalInput").ap()
    motion = nc.dram_tensor("motion", [C, T2 * V], bf16, kind="ExternalInput").ap()
    w_iv = nc.dram_tensor("w_iv", [C, C], f32, kind="ExternalInput").ap()
    b_iv = nc.dram_tensor("b_iv", [C], f32, kind="ExternalInput").ap()
    w_iq = nc.dram_tensor("w_iq", [REL, C], f32, kind="ExternalInput").ap()
    b_iq = nc.dram_tensor("b_iq", [REL], f32, kind="ExternalInput").ap()
    w_mq = nc.dram_tensor("w_mq", [REL, C], f32, kind="ExternalInput").ap()
    b_mq = nc.dram_tensor("b_mq", [REL], f32, kind="ExternalInput").ap()
    w_att = nc.dram_tensor("w_att", [C, REL], f32, kind="ExternalInput").ap()
    b_att = nc.dram_tensor("b_att", [C], f32, kind="ExternalInput").ap()
    y = nc.dram_tensor("y", [C, T1, HW], bf16, kind="ExternalOutput").ap()

    with tile.TileContext(nc) as tc, ExitStack() as ctx:
        const = ctx.enter_context(tc.tile_pool(name="const", bufs=1))
        img_pool = ctx.enter_context(tc.tile_pool(name="imgp", bufs=2))
        stg_pool = ctx.enter_context(tc.tile_pool(name="stgp", bufs=6))
        y_pool = ctx.enter_context(tc.tile_pool(name="yp", bufs=6))
        ps_pool = ctx.enter_context(tc.tile_pool(name="ps", bufs=3, space="PSUM"))
        ps_small = ctx.enter_context(tc.tile_pool(name="pss", bufs=2, space="PSUM"))

        # ---------------- setup: constants ----------------
        ident = const.tile([128, 128], f32, name="ident", tag="ident")
        masks.make_identity(nc, ident[:])

        eye = const.tile([128, 256], f32, name="eye", tag="eye")
        nc.vector.memset(eye[:], 0.0)
        for j in range(T1):
            nc.vector.memset(eye[:, j * 17 : j * 17 + 1], 1.0)

        # biases
        b_iq_sb = const.tile([REL, 1], f32, name="b_iq_sb", tag="b_iq_sb")
        nc.sync.dma_start(b_iq_sb[:], b_iq[:])
        b_mq_sb = const.tile([REL, 1], f32, name="b_mq_sb", tag="b_mq_sb")
        nc.sync.dma_start(b_mq_sb[:], b_mq[:])
        b_iv_sb = const.tile([128, 2], f32, name="b_iv_sb", tag="b_iv_sb")
        b_att_sb = const.tile([128, 2], f32, name="b_att_sb", tag="b_att_sb")
        for h in range(2):
            nc.sync.dma_start(b_iv_sb[:, h : h + 1], b_iv[h * CH : (h + 1) * CH])
            nc.sync.dma_start(b_att_sb[:, h : h + 1], b_att[h * CH : (h + 1) * CH])
        nc.scalar.mul(b_att_sb[:], b_att_sb[:], 1.0 / T1)

        # ---------------- setup: weight transposes ----------------
        # w_iv -> lhsT_iv blocks [c'(128), d(128)] at cols (kh*2+h2)*128, bf16
        w_iv_sb = const.tile([128, 512], f32, name="w_iv_sb", tag="w_iv_sb")
        for h in range(2):
            nc.sync.dma_start(
                w_iv_sb[:, h * 256 : (h + 1) * 256], w_iv[h * CH : (h + 1) * CH, :]
            )
        lhsT_iv = const.tile([128, 512], bf16, name="lhsT_iv", tag="lhsT_iv")
        for kh in range(2):
            for h2 in range(2):
                tp = ps_small.tile([128, 256], f32, name="tp", tag="pss")
                nc.tensor.transpose(
                    tp[:, 0:128],
                    w_iv_sb[:, h2 * 256 + kh * 128 : h2 * 256 + (kh + 1) * 128],
                    ident[:],
                )
                nc.scalar.copy(
                    lhsT_iv[:, (kh * 2 + h2) * 128 : (kh * 2 + h2 + 1) * 128],
                    tp[:, 0:128],
                )

        # w_iq / w_mq -> lhsT [c'(128), r(32)] at cols kh*32, scaled by 1/HW, 1/100
        w_iq_sb = const.tile([REL, C], f32, name="w_iq_sb", tag="w_iq_sb")
        nc.sync.dma_start(w_iq_sb[:], w_iq[:])
        w_mq_sb = const.tile([REL, C], f32, name="w_mq_sb", tag="w_mq_sb")
        nc.sync.dma_start(w_mq_sb[:], w_mq[:])
        lhsT_iq = const.tile([128, 64], f32r, name="lhsT_iq", tag="lhsT_iq")
        lhsT_mq = const.tile([128, 64], f32r, name="lhsT_mq", tag="lhsT_mq")
        for kh in range(2):
            tp = ps_small.tile([128, 256], f32, name="tp", tag="pss")
            nc.tensor.transpose(
                tp[:, 0:32], w_iq_sb[:, kh * 128 : (kh + 1) * 128], ident[0:32, 0:32]
            )
            nc.scalar.mul(lhsT_iq[:, kh * 32 : (kh + 1) * 32], tp[:, 0:32], 1.0 / HW)
            tp2 = ps_small.tile([128, 256], f32, name="tp2", tag="pss")
            nc.tensor.transpose(
                tp2[:, 0:32], w_mq_sb[:, kh * 128 : (kh + 1) * 128], ident[0:32, 0:32]
            )
            nc.scalar.mul(
                lhsT_mq[:, kh * 32 : (kh + 1) * 32],
                tp2[:, 0:32],
                1.0 / ((T2 // T1) * V),
            )

        # w_att [C, REL] -> lhsT_att [r(32), c(256)]
        w_att_sb = const.tile([128, 64], f32, name="w_att_sb", tag="w_att_sb")
        for h in range(2):
            nc.sync.dma_start(
                w_att_sb[:, h * 32 : (h + 1) * 32], w_att[h * CH : (h + 1) * CH, :]
            )
        lhsT_att = const.tile([REL, 256], f32r, name="lhsT_att", tag="lhsT_att")
        for h in range(2):
            tp = ps_small.tile([128, 256], f32, name="tp", tag="pss")
            nc.tensor.transpose(
                tp[0:32, 0:128], w_att_sb[:, h * 32 : (h + 1) * 32], ident[:]
            )
            nc.scalar.copy(lhsT_att[:, h * 128 : (h + 1) * 128], tp[0:32, 0:128])

        # block-diagonal att buffer (bf16), zeroed once
        # lhsT_bd[8j+cl, 256i + 32cl + g] = att[32cl+g, i, j]
        lhsT_bd = const.tile([128, NG * 128], bf16, name="lhsT_bd", tag="lhsT_bd")
        nc.gpsimd.memset(lhsT_bd[:].bitcast(f32), 0.0)

        # ---------------- motion pooling (gpsimd queue) ----------------
        mot_pool = const.tile([128, 32], f32r, name="mot_pool", tag="mot_pool")
        mots = []
        for h in range(2):
            mot = const.tile([128, T2 * V], bf16, name="mot", tag=f"mot{h}")
            nc.gpsimd.dma_start(mot[:], motion[h * CH : (h + 1) * CH, :])
            mots.append(mot)
        for h in range(2):
            with nc.allow_low_precision(reason="f32r output, fp32 accumulate"):
                nc.vector.reduce_sum(
                    mot_pool[:, h * T1 : (h + 1) * T1],
                    mots[h][:].rearrange("p (t q) -> p t q", q=(T2 // T1) * V),
                    axis=mybir.AxisListType.X,
                )

        # ---------------- phase 1: stream image ----------------
        # V_fold[8j+cl, g*HW+v] = Vv[c,j,v] (bf16), c = 32*cl+g, cl = 4*h2+q
        V_fold = const.tile([128, NG * HW], bf16, name="V_fold", tag="V_fold")
        img_sum = const.tile([128, 32], f32r, name="img_sum", tag="img_sum")

        k_sb = const.tile([REL, T1], f32, name="k_sb", tag="k_sb")

        TB = 4  # t's per load
        for tp_i in range(T1 // TB):
            # img[p, h*TB*HW + tt*HW + v] = image[h*128+p, TB*tp_i+tt, v]
            # per-(p,h) TB*784*2 = 6272B contiguous both sides.
            img = img_pool.tile([128, 2 * TB * HW], bf16, name="img", tag="img")
            eng = nc.sync if tp_i % 2 == 0 else nc.scalar
            eng.dma_start(
                img[:].rearrange("p (h tt v) -> p h tt v", h=2, v=HW),
                image[:, TB * tp_i : TB * tp_i + TB, :].rearrange(
                    "(h p) tt v -> p h tt v", h=2
                ),
            )
            imgv = img[:].rearrange("p (h tt v) -> p h tt v", h=2, v=HW)
            for tt in range(TB):
                t = TB * tp_i + tt
                # per-(c,t) spatial sum for Q: img_sum[:, h*16+t]
                for h in range(2):
                    with nc.allow_low_precision(reason="f32r out, fp32 accumulate"):
                        nc.vector.reduce_sum(
                            img_sum[:, h * T1 + t : h * T1 + t + 1],
                            imgv[:, h, tt],
                            axis=mybir.AxisListType.X,
                        )
                stg = stg_pool.tile([128, 2 * HW], bf16, name="stg", tag="stg")
                for h2 in range(2):
                    vv = ps_pool.tile([128, 1024], f32, name="vv", tag="mm")
                    for kh in range(2):
                        for cs, pcol in ((0, 0), (392, 512)):
                            nc.tensor.matmul(
                                vv[:, pcol : pcol + 392],
                                lhsT_iv[
                                    :, (kh * 2 + h2) * 128 : (kh * 2 + h2 + 1) * 128
                                ],
                                imgv[:, kh, tt, cs : cs + 392],
                                start=(kh == 0),
                                stop=(kh == 1),
                            )
                    nc.scalar.activation(
                        stg[:, h2 * HW : (h2 + 1) * HW].rearrange(
                            "p (c x) -> p c x", x=392
                        ),
                        vv[:].rearrange("p (c x) -> p c x", x=512)[:, :, 0:392],
                        AF.Identity,
                        bias=b_iv_sb[:, h2 : h2 + 1],
                    )
                # fold (HWDGE, alternating rings): per (t, h2):
                # V_fold[8t + 4h2 + q, g*HW+v] = stg[32q+g, h2*HW+v]
                for h2 in range(2):
                    feng = nc.sync if h2 == 0 else nc.scalar
                    feng.dma_start(
                        V_fold[8 * t + 4 * h2 : 8 * t + 4 * h2 + 4].rearrange(
                            "q (g v) -> q g v", v=HW
                        ),
                        stg[:, h2 * HW : (h2 + 1) * HW].rearrange(
                            "(q g) v -> q g v", q=4
                        ),
                    )

            if tp_i == 0:
                # K-side of attention: depends only on motion; emit early so it
                # overlaps phase 1 instead of sitting in the phase boundary.
                k_ps = ps_small.tile([128, 256], f32, name="k_ps", tag="pss")
                for kh in range(2):
                    nc.tensor.matmul(
                        k_ps[0:REL, 0:T1],
                        lhsT_mq[:, kh * 32 : (kh + 1) * 32],
                        mot_pool[:, kh * T1 : (kh + 1) * T1],
                        start=(kh == 0),
                        stop=(kh == 1),
                    )
                nc.scalar.activation(
                    k_sb[:], k_ps[0:REL, 0:T1], AF.Identity, bias=b_mq_sb[:, 0:1]
                )

        # ---------------- phase 2: attention ----------------
        q_ps = ps_small.tile([128, 256], f32, name="q_ps", tag="pss")
        for kh in range(2):
            nc.tensor.matmul(
                q_ps[0:REL, 0:T1],
                lhsT_iq[:, kh * 32 : (kh + 1) * 32],
                img_sum[:, kh * T1 : (kh + 1) * T1],
                start=(kh == 0),
                stop=(kh == 1),
            )
        q_sb = const.tile([REL, T1], f32, name="q_sb", tag="q_sb")
        nc.scalar.activation(
            q_sb[:], q_ps[0:REL, 0:T1], AF.Identity, bias=b_iq_sb[:, 0:1]
        )

        # s2[r, j*16+i] = tanh(K[r,i] - Q[r,j])
        diff = const.tile([REL, 256], f32, name="diff", tag="diff")
        nc.vector.tensor_sub(
            diff[:].rearrange("p (j i) -> p j i", i=T1),
            k_sb[:].unsqueeze(1).broadcast_to((REL, T1, T1)),
            q_sb[:].unsqueeze(2).broadcast_to((REL, T1, T1)),
        )
        s2 = const.tile([REL, 256], f32r, name="s2", tag="s2")
        nc.scalar.activation(s2[:], diff[:], AF.Tanh)

        # att_e[c_loc, j*16+i] = att[c, i, j] = (w_att@s2 + b_att)/16 + I
        att_es = []
        for h in range(2):
            a_ps = ps_small.tile([128, 256], f32, name="a_ps", tag="pss")
            nc.tensor.matmul(
                a_ps[:, 0:256],
                lhsT_att[:, h * 128 : (h + 1) * 128],
                s2[:],
            )
            att_f = const.tile([128, 256], f32, name="att_f", tag=f"att_f{h}")
            nc.scalar.activation(
                att_f[:],
                a_ps[:, 0:256],
                AF.Identity,
                scale=1.0 / T1,
                bias=b_att_sb[:, h : h + 1],
            )
            att_e = const.tile([128, 256], f32, name="att_e", tag=f"att_e{h}")
            nc.vector.tensor_add(att_e[:], att_f[:], eye[:])
            att_es.append(att_e)

        # PE-transpose att to put (j,i) on partitions:
        # attT[jh][16j'+i, c] = att[c, i, 8*jh+j']   (bf16)
        attTs = []
        for jh in range(2):
            attT = const.tile([128, 256], bf16, name="attT", tag=f"attT{jh}")
            for h in range(2):
                tps = ps_small.tile([128, 256], f32, name="tps", tag="pss")
                nc.tensor.transpose(
                    tps[:, 0:128],
                    att_es[h][:, jh * 128 : (jh + 1) * 128],
                    ident[:],
                )
                nc.scalar.copy(attT[:, h * 128 : (h + 1) * 128], tps[:, 0:128])
            attTs.append(attT)

        # scatter into the block-diagonal stationary: 16 DMAs, 64B packets.
        # lhsT_bd[8(8jh+j')+cl, 256i+32cl+g] = attT[jh][16j'+i, 32cl+g]
        bd_view = lhsT_bd[:].rearrange(
            "(j cl) (i hh clp g) -> j cl hh clp i g", j=16, hh=2, clp=4, g=32
        )
        di = 0
        for h in range(2):
            for jh in range(2):
                for cl_loc in range(4):
                    cl = 4 * h + cl_loc
                    d = bd_view[8 * jh : 8 * jh + 8, cl, h, cl_loc]
                    s = attTs[jh][:, 32 * cl : 32 * cl + 32]
                    eng = nc.sync if di % 2 == 0 else nc.scalar
                    eng.dma_start(d, s)
                    di += 1

        # ---------------- phase 3: y = att_bd.T @ V_fold ----------------
        # group g stationary: cols {256i + 32cl + g} -> out m = 8i + cl
        bd_g = lhsT_bd[:].rearrange("p (i cl g) -> p g i cl", i=16, g=32)
        for g in range(NG):
            yp = ps_pool.tile([128, 1024], f32, name="yp", tag="mm")
            for cs, pcol in ((0, 0), (392, 512)):
                nc.tensor.matmul(
                    yp[:, pcol : pcol + 392],
                    bd_g[:, g],
                    V_fold[:, g * HW + cs : g * HW + cs + 392],
                )
            y_sb = y_pool.tile([128, HW], bf16, name="y_sb", tag="y_sb")
            nc.vector.tensor_copy(
                y_sb[:].rearrange("p (c x) -> p c x", x=392),
                yp[:].rearrange("p (c x) -> p c x", x=512)[:, :, 0:392],
            )
            # y[32cl + g, i, v] = y_sb[8i+cl, v]
            out_eng = nc.sync if g % 2 == 0 else nc.scalar
            out_eng.dma_start(
                y[:].rearrange("(cl g) t v -> g t cl v", g=NG)[g],
                y_sb[:],
            )

    nc.compile()
    return nc


def _get_nc():
    if "nc" not in _CACHE:
        _CACHE["nc"] = _build()
    return _CACHE["nc"]


def kernel(**inputs) -> np.ndarray:
    import ml_dtypes
    from concourse.bass_utils import run_bass_kernel_spmd

    nc = _get_nc()

    bf = ml_dtypes.bfloat16
    image = np.ascontiguousarray(np.asarray(inputs["image"]).astype(bf))
    motion = np.ascontiguousarray(np.asarray(inputs["motion"]).astype(bf))
    shared = {
        k: np.ascontiguousarray(np.asarray(inputs[k], dtype=np.float32))
        for k in ("w_iv", "b_iv", "w_iq", "b_iq", "w_mq", "b_mq", "w_att", "b_att")
    }
    in_maps = []
    for n in range(NCORES):
        m = {
            "image": image[n].reshape(C, T1, HW),
            "motion": motion[n].reshape(C, T2 * V),
        }
        m.update(shared)
        in_maps.append(m)

    res = run_bass_kernel_spmd(nc, in_maps, core_ids=list(range(NCORES)))
    out = np.stack(
        [res.results[n]["y"].astype(np.float32) for n in range(NCORES)], axis=0
    )
    return out.reshape(N, C, T1, H, W)
